# revision 59
# baseline (speedup 1.0000x reference)
"""Multi-head causal self-attention on 8 Trainium2 NeuronCores.

Sharding: batch x heads (2 batches x 4 heads per core). Each core computes
QKV for its 4 heads over its batch's 2048 tokens, runs attention, then two
8-rank AllToAll collectives (one per head-pair) redistribute per-head
outputs to sequence-sharded form for the row-parallel output projection.

Schedule: a tiny barrier collective at t~0 absorbs cross-core launch skew
(otherwise it surfaces as peer-arrival wait inside the first real
AllToAll). Attention units alternate (0,sb),(1,sb) so the exp stream (the
ACT-bound half) spreads across the projection-rich phase; pair 0 completes
one unit before the end, firing a2a0 whose ~15us window ends before the
~25us engine-stall onset that follows any collective doorbell, hidden
under the last pair-1 unit. The pair-0 cat fetch + group-select pre-run on
the sync/DVE queues; the even-chunk output projection overlaps a2a1's
window; the odd chunks pipeline chunk-wise behind their selects. The
output bias is seeded into PSUM by a contract-1 matmul so the final
evacuation is a plain copy split across DVE and ACT.

Hard-won scheduling constraints (measured, not theoretical):
 - instructions parked on a queue waiting for a collective poison
   unrelated work via shared counting semaphores: emit anything that
   waits on a collective only after ALL independent work is emitted, and
   never between two collective triggers on the GpSimd queue
 - walrus emits one LDWEIGHTS per matmul (no dedup for repeated lhsT)
 - Tile dependencies are tile-granular: splitting cat into per-chunk
   tiles is required for the select/accumulate pipeline to overlap

All matmul operands are bf16 (fp32 accumulate in PSUM). Score matmuls for
the two heads of a pair are packed into disjoint PE row-groups (K=64 each)
via tile_position so they run concurrently. Softmax denominators ride as
extra rows of the V-aug stationary ("ones trick"); per-q reciprocal on DVE.

Reference semantics (torch nn.Linear convention, y = x @ W.T):
  Q = x @ Wq.T ; K = x @ Wk.T ; V = x @ Wv.T           (split into 16 heads)
  scores = Q K^T / sqrt(64), causal-masked, softmax
  out = (softmax(scores) @ V, concat heads) @ Wo.T + bo
"""

import sys
from collections import deque
from contextlib import ExitStack

sys.path.insert(0, "/opt/trn_rl_repo")

import ml_dtypes
import numpy as np

import concourse.bass as bass  # noqa: E402
import concourse.mybir as mybir  # noqa: E402
from concourse import bacc  # noqa: E402
from concourse.bass_utils import run_bass_kernel_spmd  # noqa: E402
from concourse.tile import TileContext  # noqa: E402

B = 2
S = 2048
D = 1024
H = 16
DK = 64
N_CORES = 8
GPB = 4                     # cores per batch group
HPC = H // GPB              # heads per core = 4
EL = HPC * DK               # local embedding slice = 256
P = 128                     # partitions
SBLK = 512                  # q-block size
NQ = S // SBLK              # q-blocks per batch = 4
NKT = S // P                # k-tiles per batch = 16
ND = D // P                 # d-tiles = 8
NPAIR = HPC // 2            # head pairs per core = 2
VW = HPC * P                # v_aug columns per k-tile = 512
F32 = mybir.dt.float32
F32R = mybir.dt.float32r
BF16 = mybir.dt.bfloat16
BF = ml_dtypes.bfloat16


def _classify_mask(mask: np.ndarray):
    """Classify each (q-block, k-tile) block of the [S, S] mask.

    Returns (blocks, patterns):
      blocks[j] = list of (t, pat_idx or None, qlo) k-tiles with any valid
                  entry; qlo = first q column (block-local) with any valid k
                  (0 for the first tile so PSUM start=True covers the block).
      patterns  = float [n_pat, P, SBLK] multiplicative masks in [k, q]
                  layout for partially-valid blocks (deduplicated).
    """
    mask = np.asarray(mask).astype(bool)
    patterns = []
    pat_index = {}
    blocks = []
    for j in range(NQ):
        row = []
        sub_q = mask[j * SBLK:(j + 1) * SBLK]
        for t in range(NKT):
            sub = sub_q[:, t * P:(t + 1) * P]
            if not sub.any():
                continue
            qlo = int(np.flatnonzero(sub.any(axis=1))[0])
            if not row:
                qlo = 0  # first tile must cover the whole accumulated range
            if sub.all():
                row.append((t, None, qlo))
                continue
            patT = np.ascontiguousarray(sub.T).astype(np.float32)  # [k, q]
            key = patT.tobytes()
            if key not in pat_index:
                pat_index[key] = len(patterns)
                patterns.append(patT)
            row.append((t, pat_index[key], qlo))
        assert row, f"q-block {j} has no valid keys; unsupported mask"
        blocks.append(row)
    if not patterns:
        patterns.append(np.ones((P, SBLK), np.float32))
    return blocks, np.stack(patterns)


def _build_program(blocks, n_pat):
    nc = bacc.Bacc("TRN2", target_bir_lowering=False, debug=False,
                   num_devices=N_CORES)

    # ---- I/O (all weight/act tensors pre-tiled on the host) ------------
    # xT: [D, S] = this core's batch, transposed
    xT = nc.declare_dram_parameter("xT", [D, S], BF16, isOutput=False)
    # wq/wk/wv: [P, ND*EL]; d-tile k at cols [EL*k, EL*(k+1))
    wq = nc.declare_dram_parameter("wq", [P, ND * EL], BF16, isOutput=False)
    wk = nc.declare_dram_parameter("wk", [P, ND * EL], BF16, isOutput=False)
    wv = nc.declare_dram_parameter("wv", [P, ND * EL], BF16, isOutput=False)
    # woT: [P, ND*D]; d-tile k at cols [D*k, D*(k+1))
    woT = nc.declare_dram_parameter("woT", [P, ND * D], BF16, isOutput=False)
    bob = nc.declare_dram_parameter("bob", [1, D], BF16, isOutput=False)
    # mpat: patterns duplicated along q for the two packed heads:
    # pattern n at cols [n*2*SBLK, (n+1)*2*SBLK) = [pat | pat]
    mpat = nc.declare_dram_parameter("mpat", [P, n_pat * 2 * SBLK], BF16,
                                     isOutput=False)
    # selm: [selm | 1-selm] where selm = 1.0 on cores 0-3 (batch group 0) —
    # selects which half of the 8-rank AllToAll shards is this core's group
    selm = nc.declare_dram_parameter("selm", [P, 2 * SBLK], BF16,
                                     isOutput=False)
    out = nc.declare_dram_parameter("out", [SBLK, D], F32, isOutput=True)

    # collective bounce buffers (internal DRAM), one pair per head-pair.
    # The exchange is logically 4-rank (within a batch group) but the mesh
    # only supports the full 8-rank AllToAll, so each stage block is written
    # to both candidate shard slots (j and j+4) and the receiver selects.
    a2a_in = [nc.dram_tensor(f"a2a_in{g}", [N_CORES, P, SBLK], BF16)
              for g in range(NPAIR)]
    a2a_out = [nc.dram_tensor(f"a2a_out{g}", [N_CORES, P, SBLK], BF16)
               for g in range(NPAIR)]
    # tiny barrier collective: absorbs cross-core launch skew at t~0 so
    # the real AllToAlls later see near-zero peer-arrival skew (contents
    # irrelevant)
    bar_in = nc.dram_tensor("bar_in", [N_CORES, 4], F32)
    bar_out = nc.dram_tensor("bar_out", [N_CORES, 4], F32)
    groups = [list(range(N_CORES))]

    with TileContext(nc) as tc, ExitStack() as ctx:
        const = ctx.enter_context(tc.tile_pool(name="const", bufs=1))
        persist = ctx.enter_context(tc.tile_pool(name="persist", bufs=1))
        probs_pool = ctx.enter_context(tc.tile_pool(name="probs", bufs=7))
        small = ctx.enter_context(tc.tile_pool(name="small", bufs=2))
        osb_pool = ctx.enter_context(tc.tile_pool(name="osb", bufs=3))
        # PSUM budget: ps_qk 2x1 + ps_sc 2x2 + po 2x1 = 8 banks.
        ps_qk = ctx.enter_context(
            tc.tile_pool(name="ps_qk", bufs=2, space="PSUM"))
        ps_sc = ctx.enter_context(
            tc.tile_pool(name="ps_sc", bufs=2, space="PSUM"))
        ps_po = ctx.enter_context(
            tc.tile_pool(name="ps_po", bufs=2, space="PSUM"))

        # ---- constants / weights ----------------------------------------
        warm_w = const.tile([P, DK + 1], BF16, tag="warm_w")
        nc.vector.memset(warm_w[:], 0.0)
        warm_m = const.tile([P, SBLK], BF16, tag="warm_m")
        nc.vector.memset(warm_m[:], 0.0)

        # warm the PE clock gate during the initial DMA wait
        for r in range(8):
            wps = ps_po.tile([P, SBLK], F32, tag="po", name=f"warm{r}")
            nc.tensor.matmul(wps[0:DK + 1, :], warm_w[:], warm_m[:],
                             start=True, stop=True)
        nc.gpsimd.collective_compute(
            "AllToAll", mybir.AluOpType.bypass, replica_groups=groups,
            ins=[bar_in[:]], outs=[bar_out[:]])

        # startup DMAs: wk first (first projections), x token-block 0 on the
        # scalar queue in parallel, then the rest
        w_sb = {}
        for name, t in (("wk", wk), ("wq", wq), ("wv", wv)):
            w_sb[name] = const.tile([P, ND * EL], BF16, name=f"w_{name}",
                                    tag=f"w_{name}")
        nc.sync.dma_start(out=w_sb["wk"][:], in_=wk[:])
        xt = [const.tile([P, S], BF16, name=f"xt{k}", tag=f"xt{k}")
              for k in range(ND)]
        for k in range(ND):
            nc.sync.dma_start(out=xt[k][:, 0:SBLK],
                              in_=xT[k * P:(k + 1) * P, 0:SBLK])
        nc.sync.dma_start(out=w_sb["wq"][:], in_=wq[:])
        nc.sync.dma_start(out=w_sb["wv"][:], in_=wv[:])
        for k in range(ND):
            nc.sync.dma_start(out=xt[k][:, SBLK:S],
                              in_=xT[k * P:(k + 1) * P, SBLK:S])
        mpat_sb = const.tile([P, n_pat * 2 * SBLK], BF16, tag="mpat")
        nc.sync.dma_start(out=mpat_sb[:], in_=mpat[:])
        bob_sb = const.tile([1, D], BF16, tag="bob")
        nc.sync.dma_start(out=bob_sb[:], in_=bob[:])
        ones1 = const.tile([1, P], BF16, tag="ones1")
        nc.vector.memset(ones1[:], 1.0)
        woT_sb = const.tile([P, ND * D], BF16, tag="woT")
        nc.sync.dma_start(out=woT_sb[:], in_=woT[:])

        # ---- persistent activations -------------------------------------
        # qT/kT[g]: [2*DK, S] for head pair g (rows 0:64 head 2g, 64:128
        # head 2g+1)
        qT = [persist.tile([P, S], BF16, tag=f"qT{g}", name=f"qT{g}")
              for g in range(NPAIR)]
        kT = [persist.tile([P, S], BF16, tag=f"kT{g}", name=f"kT{g}")
              for g in range(NPAIR)]
        # v_aug: [128 tokens, NKT * (4 heads * 128)]; per k-tile t, head hl:
        # cols [.., +64) = V, cols [+64, +128) = 1.0 — the 64 ones columns
        # replicate the softmax denominator into po rows 64:128 for free
        # (matmul cost depends only on the moving free dim), killing the
        # cross-partition broadcast in the normalization
        v_aug = persist.tile([P, NKT * VW], BF16, tag="vaug")
        va4 = v_aug[:].rearrange("p (t h e) -> p t h e", t=NKT, h=HPC)
        # stage[g]: normalized attention outputs [2*DK, NQ*SBLK]
        stage = [persist.tile([P, NQ * SBLK], BF16, tag=f"stg{g}",
                              name=f"stg{g}")
                 for g in range(NPAIR)]
        cat = [persist.tile([P, SBLK], BF16, tag=f"cat{i}", name=f"cat{i}")
               for i in range(ND)]
        catAB = [persist.tile([P, SBLK], BF16, tag=f"catA{i}",
                              name=f"catA{i}")
                 for i in range(ND)]
        selm_sb = const.tile([P, 2 * SBLK], BF16, tag="selm")
        nc.sync.dma_start(out=selm_sb[:], in_=selm[:])

        mpat4 = mpat_sb[:].rearrange("p (n s q) -> p n s q", n=n_pat, s=2)

        # ---- emission helpers -------------------------------------------
        def proj_qk(name, g, sb):
            """project K or Q for head pair g, token block sb."""
            dest = qT[g] if name == "wq" else kT[g]
            ps = ps_qk.tile([P, SBLK], F32, tag="ps_qk",
                            name=f"ps_{name}{g}_{sb}")
            for k in range(ND):
                nc.tensor.matmul(
                    ps[:], w_sb[name][:, bass.ds(EL * k + P * g, P)],
                    xt[k][:, bass.ts(sb, SBLK)],
                    start=(k == 0), stop=(k == ND - 1))
            nc.vector.tensor_copy(dest[:, bass.ts(sb, SBLK)], ps[:])

        def proj_v(tt):
            """project V token-major for k-tile tt (128 tokens)."""
            ps = ps_qk.tile([P, SBLK], F32, tag="ps_qk", name=f"ps_v{tt}")
            for k in range(ND):
                nc.tensor.matmul(
                    ps[:, 0:EL], xt[k][:, bass.ts(tt, P)],
                    w_sb["wv"][:, bass.ts(k, EL)],
                    start=(k == 0), stop=(k == ND - 1))
            nc.vector.tensor_copy(
                va4[:, tt, :, 0:DK],
                ps[:, 0:EL].rearrange("p (h e) -> p h e", h=HPC))
            nc.vector.memset(va4[:, tt, :, DK:P], 1.0)

        pend = deque()
        LAG = 5

        def emit_scores(it, t, pat, qlo):
            """one k-tile, both heads of pair g: packed row-tile matmuls.

            The whole chain (scores, exp, mask, and downstream attn@V) is
            trimmed to the valid q range [qlo:SBLK]; columns below qlo are
            never read, so stale data there is harmless. For shallow trims
            (qlo=128) a single full-width exp is cheaper than two split
            ones - the full-width mask still zeroes the junk."""
            g, j = it["g"], it["j"]
            w = SBLK - qlo
            ps = ps_sc.tile([P, 2 * SBLK], F32, tag="ps_sc")
            for e in range(2):
                hsl = bass.ds(DK * e, DK)
                nc.tensor.matmul(
                    ps[:, bass.ds(e * SBLK + qlo, w)],
                    kT[g][hsl, bass.ts(t, P)],
                    qT[g][hsl, bass.ds(j * SBLK + qlo, w)],
                    start=True, stop=True)
            pr = probs_pool.tile([P, 2 * SBLK], BF16, tag="probs")
            if qlo >= 256:
                for e in range(2):
                    sl = bass.ds(e * SBLK + qlo, w)
                    nc.scalar.activation(pr[:, sl], ps[:, sl],
                                         mybir.ActivationFunctionType.Exp)
                    nc.vector.tensor_mul(
                        pr[:, sl], pr[:, sl],
                        mpat_sb[:, bass.ds(2 * SBLK * pat + e * SBLK + qlo,
                                           w)])
            else:
                nc.scalar.activation(pr[:], ps[:],
                                     mybir.ActivationFunctionType.Exp)
                if pat is not None:
                    nc.vector.tensor_mul(pr[:], pr[:],
                                         mpat_sb[:, bass.ts(pat, 2 * SBLK)])
            return pr

        def emit_attnv(it, t, qlo, pr):
            g, j = it["g"], it["j"]
            if it["po"] is None:
                it["po"] = [ps_po.tile([P, SBLK], F32, tag="po",
                                       name=f"po{g}_{j}_{e}")
                            for e in range(2)]
            it["n_mm"] += 1
            first = it["n_mm"] == 1
            last = it["n_mm"] == it["total"]
            w = SBLK - qlo
            for e in range(2):
                nc.tensor.matmul(
                    it["po"][e][:, qlo:SBLK],
                    va4[:, t, 2 * g + e, :],
                    pr[:, bass.ds(e * SBLK + qlo, w)],
                    start=first, stop=last)
            if last:
                emit_norm(it)

        def emit_norm(it):
            g, j = it["g"], it["j"]
            for e in range(2):
                po = it["po"][e]
                rcp = small.tile([DK, SBLK], F32, tag="rcp")
                nc.vector.tensor_copy(rcp[:], po[DK:P, :])
                nc.vector.reciprocal_approx_fast(out=rcp[:], in_=rcp[:])
                nc.vector.tensor_mul(
                    stage[g][DK * e:DK * (e + 1), bass.ts(j, SBLK)],
                    po[0:DK, :], rcp[:])
            nc.sync.dma_start(out=a2a_in[g][j],
                              in_=stage[g][:, bass.ts(j, SBLK)])
            if g == 0 and j == NQ - 1:
                # last slot written later: delays the a2a0 doorbell until
                # ~two pair-1 units remain, so the collective (whose active
                # window stalls the engines ~25us after the doorbell) ends
                # just as attention does
                return
            nc.sync.dma_start(out=a2a_in[g][GPB + j],
                              in_=stage[g][:, bass.ts(j, SBLK)])

        def unit_gen(g, j):
            tiles = blocks[j]
            it = {"g": g, "j": j, "po": None, "n_mm": 0,
                  "total": len(tiles)}
            for (t, pat, qlo) in tiles:
                pr = emit_scores(it, t, pat, qlo)
                pend.append((it, t, qlo, pr))
                if len(pend) > LAG:
                    emit_attnv(*pend.popleft())
                yield

        def drain():
            while pend:
                emit_attnv(*pend.popleft())

        gens = deque()

        def tick(n):
            for _ in range(n):
                while gens:
                    try:
                        next(gens[0])
                        break
                    except StopIteration:
                        gens.popleft()

        def dma_cat_one(p, g):
            # chunk k=2p+g <- group-peer p's pair-g dims for my token block;
            # peers sit at global ranks p (group 0) or p+4 (group 1): fetch
            # both candidates, select by this core's group mask.
            k = 2 * p + g
            nc.sync.dma_start(out=catAB[k][:], in_=a2a_out[g][p])
            nc.sync.dma_start(out=cat[k][:], in_=a2a_out[g][GPB + p])

        def sel_cat_one(p, g, e1):
            k = 2 * p + g
            e1.tensor_mul(catAB[k][:], catAB[k][:], selm_sb[:, 0:SBLK])
            e1.tensor_mul(cat[k][:], cat[k][:], selm_sb[:, SBLK:2 * SBLK])
            e1.tensor_add(cat[k][:], cat[k][:], catAB[k][:])

        # ---- master schedule --------------------------------------------
        # Pair-0 attention completes first so a2a0 fires at ~55% of the
        # span; its transfer, cat fetch and group-select (GpSimd queue) all
        # hide under pair-1 attention. Pair-1 projections interleave with
        # pair-0 attention. The tail is a2a1 overlapped by the even-chunk
        # output projection, then the odd chunks.
        # Units alternate (0,sb),(1,sb) so the exp stream (the ACT-bound
        # half of attention) spreads across the projection-rich phase;
        # only the last 16-tile unit (1,3) runs after the a2a0 doorbell,
        # whose ~14us window then ends before the ~25us engine-stall
        # onset that follows any collective doorbell.
        for sb in range(NQ):
            for thunk in ([lambda sb=sb: proj_qk("wk", 0, sb),
                           lambda sb=sb: proj_qk("wq", 0, sb)]
                          + [lambda tt=tt: proj_v(tt)
                             for tt in range(4 * sb, 4 * sb + 4)]
                          + [lambda sb=sb: proj_qk("wk", 1, sb),
                             lambda sb=sb: proj_qk("wq", 1, sb)]):
                thunk()
                tick(2)
            gens.append(unit_gen(0, sb))
            if sb < 2:
                gens.append(unit_gen(1, sb))
        while gens:
            tick(1)
        drain()
        # deferred 8th a2a0 slot (see emit_norm): doorbell fires once the
        # sync queue reaches this, i.e. at pair-0 completion
        nc.sync.dma_start(out=a2a_in[0][GPB + NQ - 1],
                          in_=stage[0][:, bass.ts(NQ - 1, SBLK)])
        nc.gpsimd.collective_compute(
            "AllToAll", mybir.AluOpType.bypass, replica_groups=groups,
            ins=[a2a_in[0][:]], outs=[a2a_out[0][:]])

        gens.append(unit_gen(1, 2))
        gens.append(unit_gen(1, 3))
        while gens:
            tick(1)
        drain()
        nc.gpsimd.collective_compute(
            "AllToAll", mybir.AluOpType.bypass, replica_groups=groups,
            ins=[a2a_in[1][:]], outs=[a2a_out[1][:]])

        # ---- output projection ------------------------------------------
        # 8 psum slots [128 tok, 512 e] for (st, eb), reusing all pools:
        # st 0/1 -> ps_sc bufs (2 banks each), st 2 -> ps_qk, st 3 -> po.
        # cat/select emitted only now: their waits on the collectives must
        # not park in front of attention-critical queue entries.
        pss = []
        for st in range(2):
            tl = ps_sc.tile([P, 2 * SBLK], F32, tag="ps_sc",
                            name=f"ps_f{st}")
            pss.append([tl[:, 0:SBLK], tl[:, SBLK:2 * SBLK]])
        pss.append([ps_qk.tile([P, SBLK], F32, tag="ps_qk",
                               name=f"ps_f2_{eb}")[:]
                    for eb in range(2)])
        pss.append([ps_po.tile([P, SBLK], F32, tag="po",
                               name=f"ps_f3_{eb}")[:]
                    for eb in range(2)])

        # bias rides PSUM: a contract-1 matmul seeds every token row with
        # bo (start=True), so the final PSUM->SBUF evacuation is a plain
        # copy splittable across DVE and ACT.
        for st in range(SBLK // P):
            for eb in range(D // SBLK):
                nc.tensor.matmul(pss[st][eb], ones1[:],
                                 bob_sb[:, bass.ts(eb, SBLK)],
                                 start=True, stop=False)
        # even chunks (pair-0 dims, a2a0 done long ago): fetch, select and
        # accumulate chunk-wise - all overlapped by late pair-1 attention
        for p in range(GPB):
            dma_cat_one(p, 0)
            sel_cat_one(p, 0, nc.vector)
            k = 2 * p
            for st in range(SBLK // P):
                for eb in range(D // SBLK):
                    nc.tensor.matmul(
                        pss[st][eb], cat[k][:, bass.ts(st, P)],
                        woT_sb[:, bass.ds(D * k + SBLK * eb, SBLK)],
                        start=False, stop=False)
        # odd chunks: fetch on sync + DVE queues (everything behind the
        # parked DVE DMAs already depends on them), then select+accumulate
        # chunk-wise so the first accumulation starts right after the
        # first select; selects alternate DVE / GpSimd (GpSimd is safe
        # here - its queue head, the a2a1 completion wait, releases first)
        for p in range(GPB):
            dma_cat_one(p, 1)
        for p in range(GPB):
            k = 2 * p + 1
            sel_cat_one(p, 1, nc.vector if p % 2 == 0 else nc.gpsimd)
            for st in range(SBLK // P):
                for eb in range(D // SBLK):
                    nc.tensor.matmul(
                        pss[st][eb], cat[k][:, bass.ts(st, P)],
                        woT_sb[:, bass.ds(D * k + SBLK * eb, SBLK)],
                        start=False, stop=(k == 7))
        # evacuation+store pipelines st-wise behind the k=7 accumulation;
        # two evac engines and two store queues run in parallel
        for st in range(SBLK // P):
            for eb in range(D // SBLK):
                ot = osb_pool.tile([P, SBLK], F32, tag="osb")
                if eb == 0:
                    nc.vector.tensor_copy(ot[:], pss[st][eb])
                else:
                    nc.scalar.copy(ot[:], pss[st][eb])
                nc.sync.dma_start(
                    out=out[st * P:(st + 1) * P,
                            eb * SBLK:(eb + 1) * SBLK],
                    in_=ot[:])

    nc.compile()
    return nc


def _sbuf_tiled(wT):
    # [D, E] -> [P, ND*E]: row p holds d-tile k at cols [k*E, (k+1)*E)
    dd, e = wT.shape
    return np.ascontiguousarray(
        wT.reshape(dd // P, P, e).transpose(1, 0, 2).reshape(P, -1))


def _prepare_inputs(x, Wq, Wk, Wv, Wo, bo, patterns):
    x = np.asarray(x, np.float32)
    woT = _sbuf_tiled(
        np.ascontiguousarray(np.asarray(Wo, np.float32).T)).astype(BF)
    bo2 = np.asarray(bo, np.float32).reshape(1, D).astype(BF)
    scale = np.float32(1.0 / np.sqrt(DK))
    n_pat = patterns.shape[0]
    # [n_pat, P, SBLK] -> [P, n_pat * 2*SBLK] with each pattern doubled
    mpat2 = np.ascontiguousarray(
        np.concatenate([patterns, patterns], axis=2)
        .transpose(1, 0, 2).reshape(P, n_pat * 2 * SBLK)).astype(BF)
    xTb = [np.ascontiguousarray(x[b].T).astype(BF) for b in range(B)]
    wqT = np.asarray(Wq, np.float32).T * scale
    wkT = np.asarray(Wk, np.float32).T
    wvT = np.asarray(Wv, np.float32).T
    selms = [np.concatenate([np.full((P, SBLK), 1.0 - gb, BF),
                             np.full((P, SBLK), float(gb), BF)], axis=1)
             for gb in range(2)]
    in_maps = []
    for c in range(N_CORES):
        cols = slice((c % GPB) * EL, (c % GPB + 1) * EL)
        in_maps.append({
            "xT": xTb[c // GPB],
            "wq": _sbuf_tiled(wqT[:, cols]).astype(BF),
            "wk": _sbuf_tiled(wkT[:, cols]).astype(BF),
            "wv": _sbuf_tiled(wvT[:, cols]).astype(BF),
            "woT": woT,
            "bob": bo2,
            "mpat": mpat2,
            "selm": selms[c // GPB],
        })
    return in_maps


def _run(inputs, trace=False):
    blocks, patterns = _classify_mask(inputs["mask"])
    nc = _build_program(blocks, patterns.shape[0])
    in_maps = _prepare_inputs(inputs["x"], inputs["Wq"], inputs["Wk"],
                              inputs["Wv"], inputs["Wo"], inputs["bo"],
                              patterns)
    res = run_bass_kernel_spmd(nc, in_maps, list(range(N_CORES)),
                               trace=trace)
    full = np.empty((B, S, D), np.float32)
    for c in range(N_CORES):
        b, p = divmod(c, GPB)
        full[b, p * SBLK:(p + 1) * SBLK, :] = res.results[c]["out"]
    return full, res


def kernel(**inputs) -> np.ndarray:
    out, _ = _run(inputs, trace=False)
    return out



# revision 61
# speedup vs baseline: 1.0248x; 1.0248x over previous
"""Multi-head causal self-attention on 8 Trainium2 NeuronCores.

Sharding: batch x heads (2 batches x 4 heads per core). Each core computes
QKV for its 4 heads over its batch's 2048 tokens, runs attention, then two
8-rank AllToAll collectives (one per head-pair) redistribute per-head
outputs to sequence-sharded form for the row-parallel output projection.

Schedule: a tiny barrier collective at t~0 absorbs cross-core launch skew
(otherwise it surfaces as peer-arrival wait inside the first real
AllToAll). Attention units alternate (0,sb),(1,sb) so the exp stream (the
ACT-bound half) spreads across the projection-rich phase; pair 0 completes
one unit before the end, firing a2a0 whose ~15us window ends before the
~25us engine-stall onset that follows any collective doorbell, hidden
under the last pair-1 unit. The pair-0 cat fetch + group-select pre-run on
the sync/DVE queues; the even-chunk output projection overlaps a2a1's
window; the odd chunks pipeline chunk-wise behind their selects. The
output bias is seeded into PSUM by a contract-1 matmul so the final
evacuation is a plain copy split across DVE and ACT.

Hard-won scheduling constraints (measured, not theoretical):
 - instructions parked on a queue waiting for a collective poison
   unrelated work via shared counting semaphores: emit anything that
   waits on a collective only after ALL independent work is emitted, and
   never between two collective triggers on the GpSimd queue
 - walrus emits one LDWEIGHTS per matmul (no dedup for repeated lhsT)
 - Tile dependencies are tile-granular: splitting cat into per-chunk
   tiles is required for the select/accumulate pipeline to overlap

All matmul operands are bf16 (fp32 accumulate in PSUM). Score matmuls for
the two heads of a pair are packed into disjoint PE row-groups (K=64 each)
via tile_position so they run concurrently. Softmax denominators ride as
extra rows of the V-aug stationary ("ones trick"); per-q reciprocal on DVE.

Reference semantics (torch nn.Linear convention, y = x @ W.T):
  Q = x @ Wq.T ; K = x @ Wk.T ; V = x @ Wv.T           (split into 16 heads)
  scores = Q K^T / sqrt(64), causal-masked, softmax
  out = (softmax(scores) @ V, concat heads) @ Wo.T + bo
"""

import sys
from collections import deque
from contextlib import ExitStack

sys.path.insert(0, "/opt/trn_rl_repo")

import ml_dtypes
import numpy as np

import concourse.bass as bass  # noqa: E402
import concourse.mybir as mybir  # noqa: E402
from concourse import bacc  # noqa: E402
from concourse.bass_utils import run_bass_kernel_spmd  # noqa: E402
from concourse.tile import TileContext  # noqa: E402

B = 2
S = 2048
D = 1024
H = 16
DK = 64
N_CORES = 8
GPB = 4                     # cores per batch group
HPC = H // GPB              # heads per core = 4
EL = HPC * DK               # local embedding slice = 256
P = 128                     # partitions
SBLK = 512                  # q-block size
NQ = S // SBLK              # q-blocks per batch = 4
NKT = S // P                # k-tiles per batch = 16
ND = D // P                 # d-tiles = 8
NPAIR = HPC // 2            # head pairs per core = 2
VW = HPC * P                # v_aug columns per k-tile = 512
F32 = mybir.dt.float32
F32R = mybir.dt.float32r
BF16 = mybir.dt.bfloat16
BF = ml_dtypes.bfloat16


def _classify_mask(mask: np.ndarray):
    """Classify each (q-block, k-tile) block of the [S, S] mask.

    Returns (blocks, patterns):
      blocks[j] = list of (t, pat_idx or None, qlo) k-tiles with any valid
                  entry; qlo = first q column (block-local) with any valid k
                  (0 for the first tile so PSUM start=True covers the block).
      patterns  = float [n_pat, P, SBLK] multiplicative masks in [k, q]
                  layout for partially-valid blocks (deduplicated).
    """
    mask = np.asarray(mask).astype(bool)
    patterns = []
    pat_index = {}
    blocks = []
    for j in range(NQ):
        row = []
        sub_q = mask[j * SBLK:(j + 1) * SBLK]
        for t in range(NKT):
            sub = sub_q[:, t * P:(t + 1) * P]
            if not sub.any():
                continue
            qlo = int(np.flatnonzero(sub.any(axis=1))[0])
            if not row:
                qlo = 0  # first tile must cover the whole accumulated range
            if sub.all():
                row.append((t, None, qlo))
                continue
            patT = np.ascontiguousarray(sub.T).astype(np.float32)  # [k, q]
            key = patT.tobytes()
            if key not in pat_index:
                pat_index[key] = len(patterns)
                patterns.append(patT)
            row.append((t, pat_index[key], qlo))
        assert row, f"q-block {j} has no valid keys; unsupported mask"
        blocks.append(row)
    if not patterns:
        patterns.append(np.ones((P, SBLK), np.float32))
    return blocks, np.stack(patterns)


def _build_program(blocks, n_pat):
    nc = bacc.Bacc("TRN2", target_bir_lowering=False, debug=False,
                   num_devices=N_CORES)

    # ---- I/O (all weight/act tensors pre-tiled on the host) ------------
    # xT: [D, S] = this core's batch, transposed
    xT = nc.declare_dram_parameter("xT", [D, S], BF16, isOutput=False)
    # wq/wk/wv: [P, ND*EL]; d-tile k at cols [EL*k, EL*(k+1))
    wq = nc.declare_dram_parameter("wq", [P, ND * EL], BF16, isOutput=False)
    wk = nc.declare_dram_parameter("wk", [P, ND * EL], BF16, isOutput=False)
    wv = nc.declare_dram_parameter("wv", [P, ND * EL], BF16, isOutput=False)
    # woT: [P, ND*D]; d-tile k at cols [D*k, D*(k+1))
    woT = nc.declare_dram_parameter("woT", [P, ND * D], BF16, isOutput=False)
    bob = nc.declare_dram_parameter("bob", [1, D], BF16, isOutput=False)
    # mpat: patterns duplicated along q for the two packed heads:
    # pattern n at cols [n*2*SBLK, (n+1)*2*SBLK) = [pat | pat]
    mpat = nc.declare_dram_parameter("mpat", [P, n_pat * 2 * SBLK], BF16,
                                     isOutput=False)
    # selm: [selm | 1-selm] where selm = 1.0 on cores 0-3 (batch group 0) —
    # selects which half of the 8-rank AllToAll shards is this core's group
    selm = nc.declare_dram_parameter("selm", [P, 2 * SBLK], BF16,
                                     isOutput=False)
    out = nc.declare_dram_parameter("out", [SBLK, D], F32, isOutput=True)

    # collective bounce buffers (internal DRAM), one pair per head-pair.
    # The exchange is logically 4-rank (within a batch group) but the mesh
    # only supports the full 8-rank AllToAll, so each stage block is written
    # to both candidate shard slots (j and j+4) and the receiver selects.
    a2a_in = [nc.dram_tensor(f"a2a_in{g}", [N_CORES, P, SBLK], BF16)
              for g in range(NPAIR)]
    a2a_out = [nc.dram_tensor(f"a2a_out{g}", [N_CORES, P, SBLK], BF16)
               for g in range(NPAIR)]
    # tiny barrier collective: absorbs cross-core launch skew at t~0 so
    # the real AllToAlls later see near-zero peer-arrival skew (contents
    # irrelevant)
    bar_in = nc.dram_tensor("bar_in", [N_CORES, 4], F32)
    bar_out = nc.dram_tensor("bar_out", [N_CORES, 4], F32)
    groups = [list(range(N_CORES))]

    with TileContext(nc) as tc, ExitStack() as ctx:
        const = ctx.enter_context(tc.tile_pool(name="const", bufs=1))
        persist = ctx.enter_context(tc.tile_pool(name="persist", bufs=1))
        probs_pool = ctx.enter_context(tc.tile_pool(name="probs", bufs=7))
        small = ctx.enter_context(tc.tile_pool(name="small", bufs=2))
        osb_pool = ctx.enter_context(tc.tile_pool(name="osb", bufs=3))
        # PSUM budget: ps_qk 2x1 + ps_sc 2x2 + po 2x1 = 8 banks.
        ps_qk = ctx.enter_context(
            tc.tile_pool(name="ps_qk", bufs=2, space="PSUM"))
        ps_sc = ctx.enter_context(
            tc.tile_pool(name="ps_sc", bufs=2, space="PSUM"))
        ps_po = ctx.enter_context(
            tc.tile_pool(name="ps_po", bufs=2, space="PSUM"))

        # ---- constants / weights ----------------------------------------
        warm_w = const.tile([P, DK + 1], BF16, tag="warm_w")
        nc.vector.memset(warm_w[:], 0.0)
        warm_m = const.tile([P, SBLK], BF16, tag="warm_m")
        nc.vector.memset(warm_m[:], 0.0)

        # warm the PE clock gate during the initial DMA wait
        for r in range(8):
            wps = ps_po.tile([P, SBLK], F32, tag="po", name=f"warm{r}")
            nc.tensor.matmul(wps[0:DK + 1, :], warm_w[:], warm_m[:],
                             start=True, stop=True)
        nc.gpsimd.collective_compute(
            "AllToAll", mybir.AluOpType.bypass, replica_groups=groups,
            ins=[bar_in[:]], outs=[bar_out[:]])

        # startup DMAs: wk first (first projections), x token-block 0 on the
        # scalar queue in parallel, then the rest
        w_sb = {}
        for name, t in (("wk", wk), ("wq", wq), ("wv", wv)):
            w_sb[name] = const.tile([P, ND * EL], BF16, name=f"w_{name}",
                                    tag=f"w_{name}")
        nc.sync.dma_start(out=w_sb["wk"][:], in_=wk[:])
        xt = [const.tile([P, S], BF16, name=f"xt{k}", tag=f"xt{k}")
              for k in range(ND)]
        for k in range(ND):
            nc.sync.dma_start(out=xt[k][:, 0:SBLK],
                              in_=xT[k * P:(k + 1) * P, 0:SBLK])
        nc.sync.dma_start(out=w_sb["wq"][:], in_=wq[:])
        nc.sync.dma_start(out=w_sb["wv"][:], in_=wv[:])
        for k in range(ND):
            nc.sync.dma_start(out=xt[k][:, SBLK:S],
                              in_=xT[k * P:(k + 1) * P, SBLK:S])
        mpat_sb = const.tile([P, n_pat * 2 * SBLK], BF16, tag="mpat")
        nc.sync.dma_start(out=mpat_sb[:], in_=mpat[:])
        bob_sb = const.tile([1, D], BF16, tag="bob")
        nc.sync.dma_start(out=bob_sb[:], in_=bob[:])
        ones1 = const.tile([1, P], BF16, tag="ones1")
        nc.vector.memset(ones1[:], 1.0)
        woT_sb = const.tile([P, ND * D], BF16, tag="woT")
        nc.sync.dma_start(out=woT_sb[:], in_=woT[:])

        # ---- persistent activations -------------------------------------
        # qT/kT[g]: [2*DK, S] for head pair g (rows 0:64 head 2g, 64:128
        # head 2g+1)
        qT = [persist.tile([P, S], BF16, tag=f"qT{g}", name=f"qT{g}")
              for g in range(NPAIR)]
        kT = [persist.tile([P, S], BF16, tag=f"kT{g}", name=f"kT{g}")
              for g in range(NPAIR)]
        # v_aug: [128 tokens, NKT * (4 heads * 128)]; per k-tile t, head hl:
        # cols [.., +64) = V, cols [+64, +128) = 1.0 — the 64 ones columns
        # replicate the softmax denominator into po rows 64:128 for free
        # (matmul cost depends only on the moving free dim), killing the
        # cross-partition broadcast in the normalization
        v_aug = persist.tile([P, NKT * VW], BF16, tag="vaug")
        va4 = v_aug[:].rearrange("p (t h e) -> p t h e", t=NKT, h=HPC)
        # stage[g]: normalized attention outputs [2*DK, NQ*SBLK]
        stage = [persist.tile([P, NQ * SBLK], BF16, tag=f"stg{g}",
                              name=f"stg{g}")
                 for g in range(NPAIR)]
        cat = [persist.tile([P, SBLK], BF16, tag=f"cat{i}", name=f"cat{i}")
               for i in range(ND)]
        catAB = [persist.tile([P, SBLK], BF16, tag=f"catA{i}",
                              name=f"catA{i}")
                 for i in range(ND)]
        selm_sb = const.tile([P, 2 * SBLK], BF16, tag="selm")
        nc.sync.dma_start(out=selm_sb[:], in_=selm[:])

        mpat4 = mpat_sb[:].rearrange("p (n s q) -> p n s q", n=n_pat, s=2)

        # ---- emission helpers -------------------------------------------
        def proj_qk(name, g, sb):
            """project K or Q for head pair g, token block sb."""
            dest = qT[g] if name == "wq" else kT[g]
            ps = ps_qk.tile([P, SBLK], F32, tag="ps_qk",
                            name=f"ps_{name}{g}_{sb}")
            for k in range(ND):
                nc.tensor.matmul(
                    ps[:], w_sb[name][:, bass.ds(EL * k + P * g, P)],
                    xt[k][:, bass.ts(sb, SBLK)],
                    start=(k == 0), stop=(k == ND - 1))
            nc.vector.tensor_copy(dest[:, bass.ts(sb, SBLK)], ps[:])

        def proj_v(tt):
            """project V token-major for k-tile tt (128 tokens)."""
            ps = ps_qk.tile([P, SBLK], F32, tag="ps_qk", name=f"ps_v{tt}")
            for k in range(ND):
                nc.tensor.matmul(
                    ps[:, 0:EL], xt[k][:, bass.ts(tt, P)],
                    w_sb["wv"][:, bass.ts(k, EL)],
                    start=(k == 0), stop=(k == ND - 1))
            nc.vector.tensor_copy(
                va4[:, tt, :, 0:DK],
                ps[:, 0:EL].rearrange("p (h e) -> p h e", h=HPC))
            nc.vector.memset(va4[:, tt, :, DK:P], 1.0)

        pend = deque()
        LAG = 5

        def emit_scores(it, t, pat, qlo):
            """one k-tile, both heads of pair g: packed row-tile matmuls.

            The whole chain (scores, exp, mask, and downstream attn@V) is
            trimmed to the valid q range [qlo:SBLK]; columns below qlo are
            never read, so stale data there is harmless. For shallow trims
            (qlo=128) a single full-width exp is cheaper than two split
            ones - the full-width mask still zeroes the junk."""
            g, j = it["g"], it["j"]
            w = SBLK - qlo
            ps = ps_sc.tile([P, 2 * SBLK], F32, tag="ps_sc")
            for e in range(2):
                hsl = bass.ds(DK * e, DK)
                nc.tensor.matmul(
                    ps[:, bass.ds(e * SBLK + qlo, w)],
                    kT[g][hsl, bass.ts(t, P)],
                    qT[g][hsl, bass.ds(j * SBLK + qlo, w)],
                    start=True, stop=True)
            pr = probs_pool.tile([P, 2 * SBLK], BF16, tag="probs")
            if qlo >= 256:
                for e in range(2):
                    sl = bass.ds(e * SBLK + qlo, w)
                    nc.scalar.activation(pr[:, sl], ps[:, sl],
                                         mybir.ActivationFunctionType.Exp)
                    nc.vector.tensor_mul(
                        pr[:, sl], pr[:, sl],
                        mpat_sb[:, bass.ds(2 * SBLK * pat + e * SBLK + qlo,
                                           w)])
            else:
                nc.scalar.activation(pr[:], ps[:],
                                     mybir.ActivationFunctionType.Exp)
                if pat is not None:
                    nc.vector.tensor_mul(pr[:], pr[:],
                                         mpat_sb[:, bass.ts(pat, 2 * SBLK)])
            return pr

        def emit_attnv(it, t, qlo, pr):
            g, j = it["g"], it["j"]
            if it["po"] is None:
                it["po"] = [ps_po.tile([P, SBLK], F32, tag="po",
                                       name=f"po{g}_{j}_{e}")
                            for e in range(2)]
            it["n_mm"] += 1
            first = it["n_mm"] == 1
            last = it["n_mm"] == it["total"]
            w = SBLK - qlo
            for e in range(2):
                nc.tensor.matmul(
                    it["po"][e][:, qlo:SBLK],
                    va4[:, t, 2 * g + e, :],
                    pr[:, bass.ds(e * SBLK + qlo, w)],
                    start=first, stop=last)
            if last:
                emit_norm(it)

        def emit_norm(it):
            g, j = it["g"], it["j"]
            for e in range(2):
                po = it["po"][e]
                rcp = small.tile([DK, SBLK], F32, tag="rcp")
                nc.vector.tensor_copy(rcp[:], po[DK:P, :])
                nc.vector.reciprocal_approx_fast(out=rcp[:], in_=rcp[:])
                nc.vector.tensor_mul(
                    stage[g][DK * e:DK * (e + 1), bass.ts(j, SBLK)],
                    po[0:DK, :], rcp[:])
            nc.sync.dma_start(out=a2a_in[g][j],
                              in_=stage[g][:, bass.ts(j, SBLK)])
            if g == 0 and j == NQ - 1:
                # last slot written later: delays the a2a0 doorbell until
                # ~two pair-1 units remain, so the collective (whose active
                # window stalls the engines ~25us after the doorbell) ends
                # just as attention does
                return
            nc.sync.dma_start(out=a2a_in[g][GPB + j],
                              in_=stage[g][:, bass.ts(j, SBLK)])

        def unit_gen(g, j):
            tiles = blocks[j]
            it = {"g": g, "j": j, "po": None, "n_mm": 0,
                  "total": len(tiles)}
            for (t, pat, qlo) in tiles:
                pr = emit_scores(it, t, pat, qlo)
                pend.append((it, t, qlo, pr))
                if len(pend) > LAG:
                    emit_attnv(*pend.popleft())
                yield

        def drain():
            while pend:
                emit_attnv(*pend.popleft())

        gens = deque()

        def tick(n):
            for _ in range(n):
                while gens:
                    try:
                        next(gens[0])
                        break
                    except StopIteration:
                        gens.popleft()

        def dma_cat_one(p, g):
            # chunk k=2p+g <- group-peer p's pair-g dims for my token block;
            # peers sit at global ranks p (group 0) or p+4 (group 1): fetch
            # both candidates, select by this core's group mask.
            k = 2 * p + g
            nc.sync.dma_start(out=catAB[k][:], in_=a2a_out[g][p])
            nc.sync.dma_start(out=cat[k][:], in_=a2a_out[g][GPB + p])

        def sel_cat_one(p, g, e1):
            k = 2 * p + g
            e1.tensor_mul(catAB[k][:], catAB[k][:], selm_sb[:, 0:SBLK])
            e1.tensor_mul(cat[k][:], cat[k][:], selm_sb[:, SBLK:2 * SBLK])
            e1.tensor_add(cat[k][:], cat[k][:], catAB[k][:])

        # ---- master schedule --------------------------------------------
        # Pair-0 attention completes first so a2a0 fires at ~55% of the
        # span; its transfer, cat fetch and group-select (GpSimd queue) all
        # hide under pair-1 attention. Pair-1 projections interleave with
        # pair-0 attention. The tail is a2a1 overlapped by the even-chunk
        # output projection, then the odd chunks.
        # Units alternate (0,sb),(1,sb) so the exp stream (the ACT-bound
        # half of attention) spreads across the projection-rich phase;
        # only the last 16-tile unit (1,3) runs after the a2a0 doorbell,
        # whose ~14us window then ends before the ~25us engine-stall
        # onset that follows any collective doorbell.
        for sb in range(NQ):
            for thunk in ([lambda sb=sb: proj_qk("wk", 0, sb),
                           lambda sb=sb: proj_qk("wq", 0, sb)]
                          + [lambda tt=tt: proj_v(tt)
                             for tt in range(4 * sb, 4 * sb + 4)]
                          + [lambda sb=sb: proj_qk("wk", 1, sb),
                             lambda sb=sb: proj_qk("wq", 1, sb)]):
                thunk()
                tick(2)
            gens.append(unit_gen(0, sb))
            if sb < NQ - 1:
                gens.append(unit_gen(1, sb))
        while gens:
            tick(1)
        drain()
        # deferred 8th a2a0 slot (see emit_norm): doorbell fires once the
        # sync queue reaches this, i.e. at pair-0 completion
        nc.sync.dma_start(out=a2a_in[0][GPB + NQ - 1],
                          in_=stage[0][:, bass.ts(NQ - 1, SBLK)])
        nc.gpsimd.collective_compute(
            "AllToAll", mybir.AluOpType.bypass, replica_groups=groups,
            ins=[a2a_in[0][:]], outs=[a2a_out[0][:]])

        gens.append(unit_gen(1, NQ - 1))
        while gens:
            tick(1)
        drain()
        nc.gpsimd.collective_compute(
            "AllToAll", mybir.AluOpType.bypass, replica_groups=groups,
            ins=[a2a_in[1][:]], outs=[a2a_out[1][:]])

        # ---- output projection ------------------------------------------
        # 8 psum slots [128 tok, 512 e] for (st, eb), reusing all pools:
        # st 0/1 -> ps_sc bufs (2 banks each), st 2 -> ps_qk, st 3 -> po.
        # cat/select emitted only now: their waits on the collectives must
        # not park in front of attention-critical queue entries.
        pss = []
        for st in range(2):
            tl = ps_sc.tile([P, 2 * SBLK], F32, tag="ps_sc",
                            name=f"ps_f{st}")
            pss.append([tl[:, 0:SBLK], tl[:, SBLK:2 * SBLK]])
        pss.append([ps_qk.tile([P, SBLK], F32, tag="ps_qk",
                               name=f"ps_f2_{eb}")[:]
                    for eb in range(2)])
        pss.append([ps_po.tile([P, SBLK], F32, tag="po",
                               name=f"ps_f3_{eb}")[:]
                    for eb in range(2)])

        # bias rides PSUM: a contract-1 matmul seeds every token row with
        # bo (start=True), so the final PSUM->SBUF evacuation is a plain
        # copy splittable across DVE and ACT.
        for st in range(SBLK // P):
            for eb in range(D // SBLK):
                nc.tensor.matmul(pss[st][eb], ones1[:],
                                 bob_sb[:, bass.ts(eb, SBLK)],
                                 start=True, stop=False)
        # even chunks (pair-0 dims, a2a0 done long ago): fetch, select and
        # accumulate chunk-wise - all overlapped by late pair-1 attention
        for p in range(GPB):
            dma_cat_one(p, 0)
            sel_cat_one(p, 0, nc.vector)
            k = 2 * p
            for st in range(SBLK // P):
                for eb in range(D // SBLK):
                    nc.tensor.matmul(
                        pss[st][eb], cat[k][:, bass.ts(st, P)],
                        woT_sb[:, bass.ds(D * k + SBLK * eb, SBLK)],
                        start=False, stop=False)
        # odd chunks: fetch on sync + DVE queues (everything behind the
        # parked DVE DMAs already depends on them), then select+accumulate
        # chunk-wise so the first accumulation starts right after the
        # first select; selects alternate DVE / GpSimd (GpSimd is safe
        # here - its queue head, the a2a1 completion wait, releases first)
        for p in range(GPB):
            dma_cat_one(p, 1)
        for p in range(GPB):
            k = 2 * p + 1
            sel_cat_one(p, 1, nc.vector if p % 2 == 0 else nc.gpsimd)
            for st in range(SBLK // P):
                for eb in range(D // SBLK):
                    nc.tensor.matmul(
                        pss[st][eb], cat[k][:, bass.ts(st, P)],
                        woT_sb[:, bass.ds(D * k + SBLK * eb, SBLK)],
                        start=False, stop=(k == 7))
        # evacuation+store pipelines st-wise behind the k=7 accumulation;
        # two evac engines and two store queues run in parallel
        for st in range(SBLK // P):
            for eb in range(D // SBLK):
                ot = osb_pool.tile([P, SBLK], F32, tag="osb")
                if eb == 0:
                    nc.vector.tensor_copy(ot[:], pss[st][eb])
                else:
                    nc.scalar.copy(ot[:], pss[st][eb])
                nc.sync.dma_start(
                    out=out[st * P:(st + 1) * P,
                            eb * SBLK:(eb + 1) * SBLK],
                    in_=ot[:])

    nc.compile()
    return nc


def _sbuf_tiled(wT):
    # [D, E] -> [P, ND*E]: row p holds d-tile k at cols [k*E, (k+1)*E)
    dd, e = wT.shape
    return np.ascontiguousarray(
        wT.reshape(dd // P, P, e).transpose(1, 0, 2).reshape(P, -1))


def _prepare_inputs(x, Wq, Wk, Wv, Wo, bo, patterns):
    x = np.asarray(x, np.float32)
    woT = _sbuf_tiled(
        np.ascontiguousarray(np.asarray(Wo, np.float32).T)).astype(BF)
    bo2 = np.asarray(bo, np.float32).reshape(1, D).astype(BF)
    scale = np.float32(1.0 / np.sqrt(DK))
    n_pat = patterns.shape[0]
    # [n_pat, P, SBLK] -> [P, n_pat * 2*SBLK] with each pattern doubled
    mpat2 = np.ascontiguousarray(
        np.concatenate([patterns, patterns], axis=2)
        .transpose(1, 0, 2).reshape(P, n_pat * 2 * SBLK)).astype(BF)
    xTb = [np.ascontiguousarray(x[b].T).astype(BF) for b in range(B)]
    wqT = np.asarray(Wq, np.float32).T * scale
    wkT = np.asarray(Wk, np.float32).T
    wvT = np.asarray(Wv, np.float32).T
    selms = [np.concatenate([np.full((P, SBLK), 1.0 - gb, BF),
                             np.full((P, SBLK), float(gb), BF)], axis=1)
             for gb in range(2)]
    in_maps = []
    for c in range(N_CORES):
        cols = slice((c % GPB) * EL, (c % GPB + 1) * EL)
        in_maps.append({
            "xT": xTb[c // GPB],
            "wq": _sbuf_tiled(wqT[:, cols]).astype(BF),
            "wk": _sbuf_tiled(wkT[:, cols]).astype(BF),
            "wv": _sbuf_tiled(wvT[:, cols]).astype(BF),
            "woT": woT,
            "bob": bo2,
            "mpat": mpat2,
            "selm": selms[c // GPB],
        })
    return in_maps


def _run(inputs, trace=False):
    blocks, patterns = _classify_mask(inputs["mask"])
    nc = _build_program(blocks, patterns.shape[0])
    in_maps = _prepare_inputs(inputs["x"], inputs["Wq"], inputs["Wk"],
                              inputs["Wv"], inputs["Wo"], inputs["bo"],
                              patterns)
    res = run_bass_kernel_spmd(nc, in_maps, list(range(N_CORES)),
                               trace=trace)
    full = np.empty((B, S, D), np.float32)
    for c in range(N_CORES):
        b, p = divmod(c, GPB)
        full[b, p * SBLK:(p + 1) * SBLK, :] = res.results[c]["out"]
    return full, res


def kernel(**inputs) -> np.ndarray:
    out, _ = _run(inputs, trace=False)
    return out

